# revision 33
# baseline (speedup 1.0000x reference)
"""Trainium2 Bass kernel for nn_AttentionDW (depthwise-conv QKV attention).

Data-parallel over batch: 8 batch elements -> 8 NeuronCores.

Per-core pipeline (one batch element, x [256, 64, 64]):
  1. depthwise 3x3 convs (stride 1 for q, stride 2 for k/v) as 9 accumulated
     diagonal-weight matmuls on the tensor engine, BN folded into the weights
  2. 1x1 pointwise convs as matmuls; bias fused into the PSUM->SBUF move
  3. attention per head in transposed layout: scores_T[t, l] = k_ch^T q_ch
     (two heads interleaved on disjoint PE row quadrants), exp on the scalar
     engine (scores are tiny -> no max subtraction), AV matmul with a ones
     column appended to v_T so Z arrives for free, normalization via approx
     reciprocal + a DMA partition_broadcast of 1/Z + DVE mult (keeping the
     broadcast off the PE removes a per-head PE-queue stall)
  4. output projection as two K=128 matmuls (head-pairs packed across the
     full 128 partitions; odd heads are DMA-shifted to partitions 64..127)

Schedule: the attention loop is software-pipelined with q production -- the
q-path conv/pointwise for chunk lc+1 is issued between the score matmuls and
the AV matmuls of chunk lc, filling the tensor engine while the scalar
engine runs the exps (the scalar engine is the #2 bottleneck at ~134 us
busy vs ~188 us for the PE). PSUM: 2-bank score tiles x2 bufs + 4
single-bank tiles for conv/pw/AV/bcast/proj. x is DMA-loaded on the
Activation HWDGE queue in parallel with weights on the SP queue.
"""

import sys

sys.path.insert(0, "/opt/trn_rl_repo")

import numpy as np

import concourse.bass as bass
import concourse.mybir as mybir
from concourse import bacc
from concourse.tile import TileContext
from concourse import bass_utils

F32 = mybir.dt.float32
F32R = mybir.dt.float32r
F16 = mybir.dt.float16

B, C, H, W = 8, 256, 64, 64
HEADS, D = 4, 64
P = 128          # partitions
CT = 2           # channel tiles (256 / 128)
NQ = H * W       # 4096 query positions
NKV = 1024       # 32*32 kv positions
LCH = 512        # l-chunk size
NLC = NQ // LCH  # 8 l chunks
EPS = 1e-5
SCALE = 256 ** (-0.5)
TT_GROUPS = [(0, 2), (2, 4), (4, 6), (6, 8)]  # t-tile groups for batched exp


def build_nc(debug=False, iters=1, skip=(), bench_io=False):
    skip = frozenset(skip)
    nc = bacc.Bacc(None, target_bir_lowering=False)

    if bench_io:
        seed_d = nc.dram_tensor("seed", [1, 4], F32, kind="ExternalInput")
        out_d = nc.dram_tensor("out", [C, 8], F32, kind="ExternalOutput")
        scratch_d = nc.dram_tensor("scratch", [C, NQ], F32, kind="Internal")
    else:
        x_d = nc.dram_tensor("x", [C, 66 * 66], F32, kind="ExternalInput")
        dw_d = {p: nc.dram_tensor(f"dwdiag_{p}", [18, P, P], F32,
                                  kind="ExternalInput")
                for p in "qkv"}
        pwT_d = nc.dram_tensor("pwT", [3, CT, P, C], F16, kind="ExternalInput")
        pb_d = nc.dram_tensor("pb", [P, 6], F32, kind="ExternalInput")
        projT_d = nc.dram_tensor("projT", [D, HEADS, CT, P], F16,
                                 kind="ExternalInput")
        projb_d = nc.dram_tensor("projb", [P, CT], F32, kind="ExternalInput")
        ident_d = nc.dram_tensor("ident", [P, D], F16, kind="ExternalInput")
        out_d = nc.dram_tensor("out", [C, NQ], F32, kind="ExternalOutput")
    if debug:
        dbg = {
            "q": nc.dram_tensor("dbg_q", [P, CT, NQ], F16, kind="ExternalOutput"),
            "k": nc.dram_tensor("dbg_k", [P, CT, NKV], F16, kind="ExternalOutput"),
            "v": nc.dram_tensor("dbg_v", [P, CT, NKV], F16, kind="ExternalOutput"),
            "exp": nc.dram_tensor("dbg_exp", [P, 8, LCH], F16, kind="ExternalOutput"),
            "av": nc.dram_tensor("dbg_av", [D, LCH], F32, kind="ExternalOutput"),
            "zr": nc.dram_tensor("dbg_zr", [1, LCH], F32, kind="ExternalOutput"),
            "outsb": nc.dram_tensor("dbg_outsb", [P, 2, NQ], F16, kind="ExternalOutput"),
            "vt": nc.dram_tensor("dbg_vt", [P, HEADS, 8, D + 1], F16, kind="ExternalOutput"),
        }

    with TileContext(nc) as tc:
        with (
            tc.tile_pool(name="wconv", bufs=1) as wconv,
            tc.tile_pool(name="wpool", bufs=1) as wpool,
            tc.tile_pool(name="xpool", bufs=1) as xpool,
            tc.tile_pool(name="ypool", bufs=4) as ypool,
            tc.tile_pool(name="qkv", bufs=1) as qkvpool,
            tc.tile_pool(name="attn", bufs=1) as attnpool,
            tc.tile_pool(name="exp", bufs=3) as exppool,
            tc.tile_pool(name="avs", bufs=4) as avspool,
            tc.tile_pool(name="zp", bufs=4) as zpool,
            tc.tile_pool(name="ps_big", bufs=2, space="PSUM") as ps_big,
            tc.tile_pool(name="ps_small", bufs=4, space="PSUM") as ps_small,
        ):
            # ---- weights / constants in SBUF ----
            dw_sb = {}
            for p in "qkv":
                t = wconv.tile([P, 18, P], F32R, name=f"dw_{p}",
                               tag=f"dw_{p}")
                if bench_io:
                    nc.vector.memset(t[:].bitcast(F32), 0.05)
                else:
                    nc.sync.dma_start(
                        t[:], dw_d[p].rearrange("t p j -> p t j").bitcast(F32R))
                dw_sb[p] = t
            pwT_sb = wpool.tile([P, 3, CT, C], F16)
            projT_sb = wpool.tile([P, 2, CT, P], F16)
            pb_sb = wpool.tile([P, 6], F32)
            projb_sb = wpool.tile([P, CT], F32)
            ident_sb = wpool.tile([P, D], F16)
            if bench_io:
                nc.vector.memset(pwT_sb[:], 0.03)
                nc.vector.memset(projT_sb[:], 0.03)
                nc.vector.memset(pb_sb[:], 0.0)
                nc.vector.memset(projb_sb[:], 0.0)
                nc.vector.memset(ident_sb[:], 0.0)
            else:
                nc.sync.dma_start(
                    pwT_sb[:], pwT_d.rearrange("p k c o -> c p k o"))
                nc.sync.dma_start(projT_sb[:], projT_d[:])
                nc.sync.dma_start(pb_sb[:], pb_d[:])
                nc.sync.dma_start(projb_sb[:], projb_d[:])
                nc.sync.dma_start(ident_sb[:], ident_d[:])
            # ones row at partition 64 feeds the per-head recip broadcast
            # matmuls
            ones_sb = wpool.tile([65, D], F16)
            nc.vector.memset(ones_sb[:], 1.0)

            # ---- x (padded on host with the zero ring) ----
            # one tile per ct so the first conv only waits for its own
            # ct's DMA (tile-granular dependency tracking)
            x_pads = [xpool.tile([P, 66, 66], F32R, name=f"xp{i}",
                                 tag=f"xp{i}") for i in range(CT)]
            if bench_io:
                for xct in range(CT):
                    nc.vector.memset(x_pads[xct][:].bitcast(F32), 0.1)
                seed_sb = wpool.tile([1, 4], F32)
                nc.sync.dma_start(seed_sb[:], seed_d[:])
                nc.vector.tensor_scalar_add(
                    pwT_sb[0:1, 0, 0, 0:4], seed_sb[:],
                    seed_sb[:, 0:1])
            else:
                # x rides the Activation HWDGE queue, in parallel with the
                # weight DMAs on the SP queue
                xr = x_d.rearrange("(t p) f -> p t f", p=P).bitcast(F32R)
                for xct in range(CT):
                    nc.scalar.dma_start(x_pads[xct][:], xr[:, xct])

            # ---- persistent activations ----
            q_sb = qkvpool.tile([P, CT, NQ], F16)
            k_sb = qkvpool.tile([P, CT, NKV], F16)
            v_sb = qkvpool.tile([P, CT, NKV], F16)
            vT_sb = attnpool.tile([P, HEADS, 8, D + 1], F16)
            nc.vector.memset(vT_sb[:, :, :, D:D + 1], 1.0)
            # out_sb[(h%2)*64+d, h//2, l]: head-pair packed for K=128 proj
            out_sb = attnpool.tile([P, 2, NQ], F16)

            def conv_chunk(p, ct, view):
                """9-tap depthwise conv chunk -> psum tile [128, 512]."""
                ps = ps_small.tile([P, LCH], F32, tag="ps_small", name="cps")
                for tap in range(9):
                    di, dj = tap // 3, tap % 3
                    nc.tensor.matmul(
                        ps[:], dw_sb[p][:, tap * 2 + ct, :], view(di, dj),
                        start=(tap == 0), stop=(tap == 8))
                yt = ypool.tile([P, LCH], F16, tag="y", name="yt")
                nc.vector.tensor_copy(yt[:], ps[:])
                return yt

            def pw_chunk(p_idx, y_tiles, mt, dst, dsts, bcol):
                """pointwise conv chunk: contract 2 ct tiles + bias."""
                ps = ps_small.tile([P, LCH], F32, tag="ps_small", name="pps")
                for kt in range(CT):
                    nc.tensor.matmul(
                        ps[:],
                        pwT_sb[:, p_idx, kt, mt * P:(mt + 1) * P],
                        y_tiles[kt][:, :], start=(kt == 0), stop=(kt == CT - 1))
                nc.vector.tensor_scalar_add(
                    dst[:, mt, dsts], ps[:], pb_sb[:, bcol:bcol + 1])

            def q_conv(lc, ct):
                i0 = lc * 8
                return conv_chunk(
                    "q", ct,
                    lambda di, dj, c=ct: x_pads[c][:, di + i0:di + i0 + 8,
                                                   dj:dj + 64])

            def q_pw_mt(lc, y_tiles, mt):
                pw_chunk(0, y_tiles, mt, q_sb,
                         slice(lc * LCH, (lc + 1) * LCH), mt)

            def q_pw(lc, y_tiles):
                for mt in range(CT):
                    q_pw_mt(lc, y_tiles, mt)

            for _it in range(iters):
                if "qpath" in skip:
                    nc.vector.memset(q_sb[:, :, 0:2], 0.01)
                if "kvpath" in skip:
                    nc.vector.memset(k_sb[:, :, 0:2], 0.01)
                    nc.vector.memset(v_sb[:, :, 0:2], 0.01)
                if "norm" in skip:
                    nc.vector.memset(out_sb[:, :, 0:2], 0.01)

                # ---- k, v paths (stride 2, 2 chunks of 512) ----
                for p_idx, p in (() if "kvpath" in skip
                                 else ((1, "k"), (2, "v"))):
                    for kc in range(2):
                        y_tiles = []
                        for ct in range(CT):
                            i0 = kc * 32
                            y_tiles.append(conv_chunk(
                                p, ct,
                                lambda di, dj, c=ct: x_pads[c][
                                    :, di + i0:di + i0 + 32:2,
                                    dj:dj + 64:2]))
                        for mt in range(CT):
                            pw_chunk(p_idx, y_tiles, mt,
                                     k_sb if p == "k" else v_sb,
                                     slice(kc * LCH, (kc + 1) * LCH),
                                     p_idx * 2 + mt)

                # ---- v transposes: v_T[t, d] per head per t-tile ----
                for h in range(0 if "vt" in skip else HEADS):
                    pp = (h % 2) * D
                    pst = ps_small.tile([P, 8, D], F16, tag="ps_small",
                                        name="pst")
                    for tt in range(8):
                        nc.tensor.transpose(
                            pst[:, tt, :],
                            v_sb[pp:pp + D, h // 2, tt * P:(tt + 1) * P],
                            ident_sb[pp:pp + D, :])
                    nc.vector.tensor_copy(vT_sb[:, h, :, 0:D], pst[:])

                # ---- q chunk 0 (prologue) ----
                if "qpath" not in skip:
                    q_pw(0, [q_conv(0, 0), q_conv(0, 1)])

                # ---- attention + pipelined q production ----
                for lc in range(NLC):
                    y_next = {}
                    for hp in range(2):
                        ct = hp
                        exp_pair = [exppool.tile([P, 8, LCH], F16, tag="exp",
                                                 name=f"exp{i}")
                                    for i in range(2)]
                        if "exp" in skip:
                            for i in range(2):
                                nc.vector.memset(exp_pair[i][:, :, 0:2], 0.01)
                        for g0, g1 in TT_GROUPS:
                            sps_pair = [ps_big.tile([P, 2, LCH], F32, tag="s",
                                                    name=f"sps{i}")
                                        for i in range(2)]
                            if "scores" in skip:
                                for i in range(2):
                                    nc.vector.memset(
                                        sps_pair[i][:, 0:1, 0:2], 0.01)
                            # interleave the two heads so the PE runs them
                            # concurrently on disjoint row groups
                            for tt in (() if "scores" in skip
                                       else range(g0, g1)):
                                for hi in range(2):
                                    pp = hi * D
                                    nc.tensor.matmul(
                                        sps_pair[hi][:, tt - g0, :],
                                        k_sb[pp:pp + D, ct,
                                             tt * P:(tt + 1) * P],
                                        q_sb[pp:pp + D, ct,
                                             lc * LCH:(lc + 1) * LCH],
                                        start=True, stop=True)
                            for hi in range(0 if "exp" in skip else 2):
                                nc.scalar.activation(
                                    exp_pair[hi][:, g0:g1, :],
                                    sps_pair[hi][:, 0:g1 - g0, :],
                                    mybir.ActivationFunctionType.Exp)
                        # PE filler while the scalar engine runs the exps:
                        # produce next chunk's q conv
                        if lc + 1 < NLC and "qpath" not in skip:
                            y_next[hp] = q_conv(lc + 1, hp)
                        for hi in range(2):
                            # extra PE filler between the AV groups of the
                            # second head pair: next chunk's first pw piece
                            if (hp == 1 and hi == 1 and lc + 1 < NLC
                                    and "qpath" not in skip):
                                q_pw_mt(lc + 1, [y_next[0], y_next[1]], 0)
                            h = 2 * ct + hi
                            exp_sb = exp_pair[hi]
                            avps = ps_small.tile([P, LCH], F32,
                                                 tag="ps_small", name="avps")
                            if "av" in skip:
                                nc.vector.memset(avps[0:D + 1, 0:2], 0.01)
                            for tt in range(0 if "av" in skip else 8):
                                nc.tensor.matmul(
                                    avps[0:D + 1, :], vT_sb[:, h, tt, :],
                                    exp_sb[:, tt, :],
                                    start=(tt == 0), stop=(tt == 7))
                            # base partition must be 0: the custom DVE op
                            # mis-lowers nonzero base partitions. Rows 0:64
                            # are junk reciprocals, never read.
                            if "norm" in skip:
                                continue
                            zr = zpool.tile([65, LCH], F32, tag="zr")
                            nc.vector.reciprocal_approx_fast(
                                zr[0:D + 1, :], avps[0:D + 1, :])
                            av_sb = avspool.tile([D, LCH], F32, tag="av")
                            nc.vector.tensor_copy(av_sb[:], avps[0:D, :])
                            # broadcast 1/Z over 64 partitions on the DMA
                            # engines: move it to partition 0, then an
                            # InstPartitionBroadcast fans it out (frees the
                            # PE K=1 matmul and the DVE fp16 staging copy)
                            zr0 = zpool.tile([1, LCH], F32, tag="zr0")
                            nc.sync.dma_start(zr0[:], zr[D:D + 1, :])
                            bps_s = avspool.tile([P, LCH], F32, tag="bps_s")
                            nc.gpsimd.partition_broadcast(bps_s[:],
                                                          zr0[0:1, :])
                            lcs = slice(lc * LCH, (lc + 1) * LCH)
                            g = h // 2
                            if h % 2 == 0:
                                nc.vector.tensor_tensor(
                                    out_sb[0:D, g, lcs],
                                    av_sb[:], bps_s[0:D, :],
                                    mybir.AluOpType.mult)
                            else:
                                # odd heads live on partitions 64..127 so the
                                # projection can contract K=128; DMA shifts
                                # av up (engines other than DMA cannot move
                                # data across partitions)
                                av2 = avspool.tile([P, LCH], F32, tag="av2")
                                nc.sync.dma_start(av2[D:P, :], av_sb[:])
                                nc.vector.tensor_tensor(
                                    out_sb[D:P, g, lcs],
                                    av2[D:P, :], bps_s[D:P, :],
                                    mybir.AluOpType.mult)
                            if debug and lc == 0 and h == 0:
                                nc.sync.dma_start(dbg["exp"][:], exp_sb[:])
                                nc.sync.dma_start(dbg["av"][:], av_sb[:])
                                nc.sync.dma_start(dbg["zr"][:],
                                                  zr[D:D + 1, :])

                    # next chunk's remaining q pointwise conv
                    if lc + 1 < NLC and "qpath" not in skip:
                        q_pw_mt(lc + 1, [y_next[0], y_next[1]], 1)

                    # ---- projection for this l chunk ----
                    for mt in range(0 if "proj" in skip else CT):
                        ps = ps_small.tile([P, LCH], F32, tag="ps_small",
                                           name="prps")
                        for g in range(2):
                            nc.tensor.matmul(
                                ps[:], projT_sb[:, g, mt, :],
                                out_sb[:, g, lc * LCH:(lc + 1) * LCH],
                                start=(g == 0), stop=(g == 1))
                        fin = ypool.tile([P, LCH], F32, tag="fin")
                        nc.vector.tensor_scalar_add(
                            fin[:], ps[:], projb_sb[:, mt:mt + 1])
                        if bench_io:
                            nc.sync.dma_start(
                                scratch_d[mt * P:(mt + 1) * P,
                                          lc * LCH:(lc + 1) * LCH], fin[:])
                            if lc == 0:
                                nc.sync.dma_start(
                                    out_d[mt * P:(mt + 1) * P, :],
                                    fin[:, 0:8])
                        else:
                            nc.sync.dma_start(
                                out_d[mt * P:(mt + 1) * P,
                                      lc * LCH:(lc + 1) * LCH],
                                fin[:])

            if debug:
                nc.sync.dma_start(dbg["q"][:], q_sb[:])
                nc.sync.dma_start(dbg["k"][:], k_sb[:])
                nc.sync.dma_start(dbg["v"][:], v_sb[:])
                nc.sync.dma_start(dbg["outsb"][:], out_sb[:])
                nc.sync.dma_start(dbg["vt"][:], vT_sb[:])

    nc.finalize()
    return nc


_NC = None


def _get_nc():
    global _NC
    if _NC is None:
        _NC = build_nc()
    return _NC


def _fold_weights(inputs):
    """Fold BN into depthwise weights; biases through the pointwise convs."""
    host = {}
    for p in "qkv":
        dw = np.asarray(inputs[f"dw_{p}"])[:, 0]          # [256, 3, 3]
        g = np.asarray(inputs[f"g_{p}"])
        bta = np.asarray(inputs[f"b_{p}"])
        mu = np.asarray(inputs[f"m_{p}"])
        var = np.asarray(inputs[f"v_{p}"])
        pw = np.asarray(inputs[f"pw_{p}"])                # [256, 256]
        inv = g / np.sqrt(var + EPS)
        dwf = (dw * inv[:, None, None]).astype(np.float32)
        pbias = (pw @ (bta - mu * inv)).astype(np.float32)
        if p == "q":
            pw = pw * SCALE
            pbias = pbias * SCALE
        host[f"dwf_{p}"] = dwf
        host[f"pw_{p}"] = pw.astype(np.float32)
        host[f"pb_{p}"] = pbias
    # diagonal matrices for the conv matmuls: [18, 128, 128], tap-major
    for p in "qkv":
        dwf = host[f"dwf_{p}"]
        diag = np.zeros((18, P, P), np.float32)
        for tap in range(9):
            di, dj = tap // 3, tap % 3
            for ct in range(CT):
                d = diag[tap * 2 + ct]
                np.fill_diagonal(d, dwf[ct * P:(ct + 1) * P, di, dj])
        host[f"dwdiag_{p}"] = diag
    host["pwT"] = np.stack(
        [host[f"pw_{p}"].T.reshape(CT, P, C) for p in "qkv"]).astype(
        np.float16)                                            # [3, 2, 128, 256]
    host["pb"] = np.stack(
        [host[f"pb_{p}"].reshape(CT, P) for p in "qkv"]).transpose(
        2, 0, 1).reshape(P, 6).astype(np.float32)         # [128, (proj, mt)]
    # proj lhsT head-pair packed: projT[(h%2)*64+d, h//2, mt, o]
    #   = proj_w[mt*128+o, h*64+d]
    pjt = np.asarray(inputs["proj_w"]).T.reshape(2, 2, D, CT, P)  # [g,par,d,mt,o]
    host["projT"] = np.ascontiguousarray(
        pjt.transpose(1, 2, 0, 3, 4).reshape(P, 2, CT, P)).astype(
        np.float16)                                            # [128, 2, 2, 128]
    host["projb"] = np.ascontiguousarray(
        np.asarray(inputs["proj_b"]).reshape(CT, P).T).astype(np.float32)
    host["ident"] = np.vstack([np.eye(D), np.eye(D)]).astype(np.float16)
    return host


def kernel(**inputs):
    nc = _get_nc()
    host = _fold_weights(inputs)
    x = np.asarray(inputs["x"]).astype(np.float32)
    common = {
        "pwT": host["pwT"], "pb": host["pb"],
        "projT": host["projT"], "projb": host["projb"],
        "ident": host["ident"],
        "dwdiag_q": host["dwdiag_q"], "dwdiag_k": host["dwdiag_k"],
        "dwdiag_v": host["dwdiag_v"],
    }
    xp = np.zeros((B, C, 66, 66), np.float32)
    xp[:, :, 1:65, 1:65] = x.reshape(B, C, H, W)
    in_maps = [
        {"x": np.ascontiguousarray(xp[b].reshape(C, 66 * 66)), **common}
        for b in range(B)
    ]
    res = bass_utils.run_bass_kernel_spmd(nc, in_maps, core_ids=list(range(B)))
    out = np.stack([r["out"].reshape(C, H, W) for r in res.results])
    return out.astype(np.float32)


if __name__ == "__main__":
    import tempfile
    nc = build_nc()
    print("build OK")
    if "--compile" in sys.argv:
        neff = bass_utils.compile_bass_kernel(nc, tempfile.mkdtemp())
        print("COMPILED:", neff)

